# revision 1
# baseline (speedup 1.0000x reference)
"""GAT (2-layer, 4-head) forward on 8 Trainium2 NeuronCores (Bass/Tile).

Sharding: destination nodes (and their incident edges, grouped by dst) are
split across 8 cores; each core owns 49 blocks of 128 dst nodes (LPT degree
balanced). Layer-1 node features h1 = x @ [W1 | A_s1 | A_d1] are computed
replicated on every core into a per-core gather table whose rows are rotated
so the core's own nodes occupy rows [0, 6272). Table rows are 512B: fp8 h1
payload + f32 a_s1. Edge aggregation gathers source rows and per-edge a_d
rows with dma_gather (int16 indices, lo/hi base split, batched over block
groups) and scatter-adds messages into a per-block PSUM via a one-hot matmul;
softmax normalization is folded in as a per-(dst,head) reciprocal scale.
Layer-2 feature rows carry a_s2 inline (512B rows) and are exchanged with a
pipelined per-group AllGather that overlaps layer-1 edge processing.
"""
import sys

sys.path.insert(0, "/opt/trn_rl_repo")

import numpy as np
import ml_dtypes

import concourse.bass as bass
import concourse.mybir as mybir
import concourse.tile as tile
from concourse import bacc
from concourse.bass_utils import run_bass_kernel_spmd

DT2 = mybir.dt.float16
DT8 = mybir.dt.float8e4
F32 = mybir.dt.float32
I16 = mybir.dt.int16
ALU = mybir.AluOpType
ACTF = mybir.ActivationFunctionType

NCORES = 8
P = 128
LO_LIMIT = 32768  # int16 index reach for dma_gather
GQ = 4   # blocks per src-gather group
AGB = 7  # blocks per AllGather group (last group is the remainder)


class Cfg:
    def __init__(self, n_nodes=50000, in_f=256, hid=64, heads=4, labels=32):
        self.N = n_nodes
        self.IN_F = in_f
        self.HID = hid
        self.HEADS = heads
        self.LABELS = labels
        self.D1 = heads * hid
        self.D2 = heads * labels
        self.own = -(-n_nodes // NCORES)
        self.NB = -(-self.own // P)
        self.own_pad = self.NB * P
        self.NPOS = NCORES * self.own_pad
        # table rows (fp8): [h fp8 (D) | a_s f32 (H) as fp8 slots | pad] -> 256B mult
        self.ROW1 = -(-(self.D1 + 4 * heads) // 256) * 256
        self.ROW2 = -(-(self.D2 + 4 * heads) // 256) * 256
        self.W1C = self.D1 + 2 * heads
        self.W2C = self.D2 + 2 * heads
        self.NAG = -(-self.NB // AGB)  # allgather groups


def _wrap_idx(idx):
    """idx [n] (n%16==0) -> [16, n//16] int16: idx i at [i%16, i//16]."""
    n = len(idx)
    return np.asarray(idx, np.int16).reshape(n // 16, 16).T


class HostPrep:
    def __init__(self, cfg, edge_index):
        self.cfg = cfg
        N = cfg.N
        ei = np.asarray(edge_index, np.int64)
        src = np.concatenate([ei[0], np.arange(N, dtype=np.int64)])
        dst = np.concatenate([ei[1], np.arange(N, dtype=np.int64)])
        deg = np.bincount(dst, minlength=N)

        # LPT: nodes by degree desc -> least-loaded core -> least-loaded block
        order = np.argsort(-deg, kind="stable")
        core_sum = np.zeros(NCORES)
        core_cnt = np.zeros(NCORES, np.int64)
        node_core = np.empty(N, np.int64)
        for n in order:
            c = int(np.argmin(np.where(core_cnt < cfg.own, core_sum, np.inf)))
            node_core[n] = c
            core_sum[c] += deg[n]
            core_cnt[c] += 1
        node_bin = np.empty(N, np.int64)
        node_slot = np.empty(N, np.int64)
        for c in range(NCORES):
            nodes = order[node_core[order] == c]
            bin_sum = np.zeros(cfg.NB)
            bin_cnt = np.zeros(cfg.NB, np.int64)
            for n in nodes:
                b = int(np.argmin(np.where(bin_cnt < P, bin_sum, np.inf)))
                node_bin[n] = b
                node_slot[n] = bin_cnt[b]
                bin_sum[b] += deg[n]
                bin_cnt[b] += 1
        gpos = node_core * cfg.own_pad + node_bin * P + node_slot
        self.node_core, self.node_bin, self.node_slot = node_core, node_bin, node_slot
        self.gpos = gpos
        # table2 global position: AG-group-major so each group's collective
        # output region is contiguous: [group][core][bin-in-group][slot]
        bg = np.array([min(AGB, cfg.NB - g * AGB) for g in range(cfg.NAG)])
        agbase = np.concatenate([[0], np.cumsum(NCORES * bg * P)])[:-1]
        self.agbase = agbase
        ag = node_bin // AGB
        g2pos = (agbase[ag] + node_core * (bg[ag] * P)
                 + (node_bin - ag * AGB) * P + node_slot)
        self.g2pos = g2pos

        ecore = node_core[dst]
        ebin = node_bin[dst]
        eloc = node_bin[dst] * P + node_slot[dst]
        edslot = node_slot[dst]
        esrc_g1 = gpos[src]
        esrc_g2 = g2pos[src]

        # per (layer, core, block): chunk caps for lo/hi parts
        self.caps = {1: [np.zeros(cfg.NB, np.int64), np.zeros(cfg.NB, np.int64)],
                     2: [np.zeros(cfg.NB, np.int64), np.zeros(cfg.NB, np.int64)]}
        groups = {}
        for c in range(NCORES):
            m = ecore == c
            s1 = (esrc_g1[m] - c * cfg.own_pad) % cfg.NPOS
            s2 = esrc_g2[m]
            bb = ebin[m]
            ll = eloc[m]
            dl = edslot[m]
            for b in range(cfg.NB):
                mb = bb == b
                for layer, s in ((1, s1[mb]), (2, s2[mb])):
                    lo = s < LO_LIMIT
                    for part, mm in ((0, lo), (1, ~lo)):
                        sv = s[mm] - (LO_LIMIT if part else 0)
                        av = ll[mb][mm]
                        dv = dl[mb][mm]
                        o = np.argsort(dv, kind="stable")
                        groups[(layer, c, b, part)] = (sv[o], av[o], dv[o])
                        self.caps[layer][part][b] = max(
                            self.caps[layer][part][b], -(-len(sv) // P))
        self.groups = groups

        lo_caps = {1: self.caps[1][0], 2: self.caps[2][0]}
        hi_caps = {1: self.caps[1][1], 2: self.caps[2][1]}
        self.ngq = -(-cfg.NB // GQ)
        # per (layer, part): src idx DRAM array [P, totcols]; per layer: ad idx
        # [P, totcols]; dl [P, nch]. Column layouts (host+program agree):
        #  src part p: for gq: for b in gq: cap[p][b] chunks
        #  ad/dl:      for gq: for b in gq: cap[0][b] then cap[1][b] chunks
        self.idx_src = {}
        self.idx_ad = {}
        self.dstloc = {}
        for layer in (1, 2):
            nch = int(lo_caps[layer].sum() + hi_caps[layer].sum())
            ncols = {0: int(lo_caps[layer].sum()) * 8,
                     1: int(hi_caps[layer].sum()) * 8}
            for c in range(NCORES):
                iw = {0: np.zeros((P, ncols[0]), np.int16),
                      1: np.zeros((P, ncols[1]), np.int16)}
                aw = np.zeros((P, nch * 8), np.int16)
                dw = np.full((P, nch), 999.0, np.float32)
                col = {0: 0, 1: 0}
                cola = 0
                ch = 0
                for b in range(cfg.NB):
                    for part in (0, 1):
                        cap = int(self.caps[layer][part][b])
                        if cap == 0:
                            continue
                        sv, av, dv = groups[(layer, c, b, part)]
                        n = cap * P
                        svp = np.zeros(n, np.int64)
                        avp = np.zeros(n, np.int64)
                        dvp = np.full(n, 999, np.int64)
                        svp[:len(sv)] = sv
                        avp[:len(av)] = (av - (b // GQ) * GQ * P
                                         if layer == 2 else av)
                        dvp[:len(dv)] = dv
                        w = np.tile(_wrap_idx(svp), (8, 1))
                        iw[part][:, col[part]:col[part] + n // 16] = w
                        aw[:, cola:cola + n // 16] = np.tile(_wrap_idx(avp), (8, 1))
                        dw[:, ch:ch + cap] = dvp.reshape(cap, P).T
                        col[part] += n // 16
                        cola += n // 16
                        ch += cap
                self.idx_src[(layer, c, 0)] = iw[0]
                self.idx_src[(layer, c, 1)] = iw[1]
                self.idx_ad[(layer, c)] = aw
                self.dstloc[(layer, c)] = dw


def build_program(cfg, prep, with_bias1, collective=True):
    nc = bacc.Bacc("TRN2", target_bir_lowering=False, debug=False,
                   num_devices=NCORES)
    H = cfg.HEADS
    D1, D2 = cfg.D1, cfg.D2
    NB, NPOS = cfg.NB, cfg.NPOS
    ROW1, ROW2 = cfg.ROW1, cfg.ROW2
    NT = NPOS // P
    K1 = cfg.IN_F // P
    K2 = D1 // P
    W1C, W2C = cfg.W1C, cfg.W2C
    NGQ = prep.ngq

    caps = {1: prep.caps[1], 2: prep.caps[2]}
    nch = {lay: int(caps[lay][0].sum() + caps[lay][1].sum()) for lay in (1, 2)}
    ncols = {(lay, part): int(caps[lay][part].sum()) * 8
             for lay in (1, 2) for part in (0, 1)}

    xT = nc.dram_tensor("xT", [cfg.IN_F, NPOS], DT2, kind="ExternalInput")
    w1e = nc.dram_tensor("w1e", [cfg.IN_F, W1C], DT2, kind="ExternalInput")
    w2e = nc.dram_tensor("w2e", [D1, W2C], DT2, kind="ExternalInput")
    bias1 = nc.dram_tensor("bias1", [1, W1C], F32, kind="ExternalInput")
    bias2 = nc.dram_tensor("bias2", [1, W2C], F32, kind="ExternalInput")
    ones1 = nc.dram_tensor("ones1", [1, P], F32, kind="ExternalInput")
    iota = nc.dram_tensor("iota", [P, P], DT2, kind="ExternalInput")
    ident = nc.dram_tensor("ident", [P, P], DT2, kind="ExternalInput")
    dz = nc.dram_tensor("dz", [P, 8], I16, kind="ExternalInput")
    is_d = {}
    for lay in (1, 2):
        for part in (0, 1):
            is_d[(lay, part)] = nc.dram_tensor(
                f"is{lay}p{part}", [P, max(ncols[(lay, part)], 16)], I16,
                kind="ExternalInput")
        is_d[(lay, "ad")] = nc.dram_tensor(
            f"ia{lay}", [P, nch[lay] * 8], I16, kind="ExternalInput")
    dl1 = nc.dram_tensor("dl1", [P, nch[1]], F32, kind="ExternalInput")
    dl2 = nc.dram_tensor("dl2", [P, nch[2]], F32, kind="ExternalInput")
    out = nc.dram_tensor("out", [cfg.own_pad, D2], F32, kind="ExternalOutput")

    with tile.TileContext(nc) as tc:
        with tc.tile_pool(name="dram", bufs=1, space="DRAM") as dram, \
             tc.tile_pool(name="const", bufs=1) as cp:
            table1 = dram.tile([NPOS, ROW1], DT8)
            ad1 = dram.tile([cfg.own_pad, P], DT2)
            ad2g = [dram.tile([min(GQ, NB - g * GQ) * P, P], DT2,
                              tag=f"ad2g{g}", name=f"ad2g{g}")
                    for g in range(prep.ngq)]
            h2sh = [dram.tile([AGB * P, ROW2], DT8, tag=f"h2sh{g}",
                              name=f"h2sh{g}")
                    for g in range(cfg.NAG)]
            table2 = dram.tile([NPOS, ROW2], DT8)

            def load_const(name, dram_t, shape, dt):
                t = cp.tile(shape, dt, tag=name, name=name + "_sb")
                nc.sync.dma_start(t[:], dram_t[:])
                return t

            iota_sb = load_const("iota", iota, [P, P], DT2)
            # warm-up gather: hoists the gpsimd library load to t~0 so real
            # gathers are not gated on it late in phase A
            dz_sb = load_const("dz", dz, [P, 8], I16)
            warm = cp.tile([P, 1, P], DT2, tag="warm", name="warm")
            nc.gpsimd.dma_gather(warm[:], iota[:], dz_sb[:], P, P, P,
                                 single_packet=False)
            ident_sb = load_const("ident", ident, [P, P], DT2)
            bias1_sb = load_const("bias1", bias1, [1, W1C], F32)
            bias2_sb = load_const("bias2", bias2, [1, W2C], F32)
            ones1_sb = load_const("ones1", ones1, [1, P], F32)
            dl1_sb = load_const("dl1", dl1, [P, nch[1]], F32)
            dl2_sb = load_const("dl2", dl2, [P, nch[2]], F32)
            w1_sb = [cp.tile([P, W1C], DT2, tag=f"w1_{k}", name=f"w1sb{k}") for k in range(K1)]
            for k in range(K1):
                nc.sync.dma_start(w1_sb[k][:], w1e[k * P:(k + 1) * P, :])
            w2_sb = [cp.tile([P, W2C], DT2, tag=f"w2_{k}", name=f"w2sb{k}") for k in range(K2)]
            for k in range(K2):
                nc.sync.dma_start(w2_sb[k][:], w2e[k * P:(k + 1) * P, :])

            # ---------------- Phase A: dense layer 1 (replicated) -----------
            SEG = 32
            with tc.tile_pool(name="dA", bufs=3) as dp, \
                 tc.tile_pool(name="dAp", bufs=4, space="PSUM") as dpp:
                RB = 8  # tiles per batched table1-row write
                RW = D1 + 4 * H
                for seg in range(0, NT, SEG):
                    ntile = min(SEG, NT - seg)
                    xs = [dp.tile([P, ntile * P], DT2, tag=f"xs{k}", name=f"xs{k}")
                          for k in range(K1)]
                    for k in range(K1):
                        nc.sync.dma_start(
                            xs[k][:],
                            xT[k * P:(k + 1) * P, seg * P:(seg + ntile) * P])
                    for t0 in range(0, ntile, RB):
                        nt = min(RB, ntile - t0)
                        # rows: [h fp8 (D1) | a_s f32 bitcast (4H fp8 slots)]
                        rows = dp.tile([P, nt, RW], DT8, tag="rows")
                        adr = None
                        for t in range(t0, t0 + nt):
                            ps = dpp.tile([P, W1C], F32, tag="ps")
                            for k in range(K1):
                                nc.tensor.matmul(
                                    ps[:], xs[k][:, t * P:(t + 1) * P], w1_sb[k][:],
                                    start=(k == 0), stop=(k == K1 - 1 and not with_bias1))
                            if with_bias1:
                                nc.tensor.matmul(ps[:], ones1_sb[:], bias1_sb[:],
                                                 start=False, stop=True)
                            j = t - t0
                            if t % 2 == 0:
                                nc.scalar.copy(rows[:, j, 0:D1], ps[:, 0:D1])
                            else:
                                nc.vector.tensor_copy(rows[:, j, 0:D1],
                                                      ps[:, 0:D1])
                            nc.vector.tensor_copy(
                                rows[:, j, D1:D1 + 4 * H].bitcast(F32),
                                ps[:, D1:D1 + H])
                            if seg + t < NB:
                                if adr is None:
                                    adr = dp.tile([P, nt, H], DT2, tag="adr")
                                nc.vector.tensor_copy(
                                    adr[:, t - t0, :], ps[:, D1 + H:D1 + 2 * H])
                        gt = seg + t0
                        nc.scalar.dma_start(
                            table1[gt * P:(gt + nt) * P, 0:RW].rearrange(
                                "(t p) c -> p t c", t=nt), rows[:])
                        if adr is not None:
                            na = min(NB - gt, nt)
                            nc.scalar.dma_start(
                                ad1[gt * P:(gt + na) * P, 0:H].rearrange(
                                    "(t p) c -> p t c", t=na), adr[:, 0:na, :])

            # ---- ad prebuild: gather 256B/edge rows, compact to H vals ----
            def group_csa(lay, gq):
                blocks = range(gq * GQ, min((gq + 1) * GQ, NB))
                return {p: int(sum(int(caps[lay][p][b]) for b in blocks))
                        for p in (0, 1)}

            # per-(layer, group) ad-idx column offsets: prebuilds may be
            # emitted out of group order, so never use a running cursor
            adoff = {}
            for lay_ in (1, 2):
                off_ = 0
                for gq_ in range(NGQ):
                    adoff[(lay_, gq_)] = off_
                    off_ += sum(group_csa(lay_, gq_).values()) * 8

            def prebuild_ad(lay, adt_g, gq, csa, gpool, cpool):
                na = csa * P
                iat = gpool.tile([P, na // 16], I16, tag="iat")
                o0 = adoff[(lay, gq)]
                nc.sync.dma_start(
                    iat[:], is_d[(lay, "ad")][:, o0:o0 + na // 16])
                adg = gpool.tile([P, csa, P], DT2, tag="adg")
                nc.gpsimd.dma_gather(
                    adg[:], adt_g[:], iat[:], na, na, P, single_packet=False)
                adc = cpool.tile([P, csa, H], DT2, tag=f"adc{lay}_{gq}",
                                 name=f"adc{lay}_{gq}")
                nc.vector.tensor_copy(adc[:], adg[:, :, 0:H])
                return adc

            # ---- Edge phase helper (shared by layer 1 / layer 2) -----------
            def edge_layer(lay, table, dl_sb, DL, postproc, bp, sp, bpp, adc,
                           interleave=None):
                """DL: payload width (D1 or D2); postproc(b, ps1)."""
                lo_c, hi_c = caps[lay]
                col = {0: 0, 1: 0}
                ch = 0
                rowlen = cfg.ROW1 if lay == 1 else cfg.ROW2
                asw = 4 * H  # a_s slice width (16B)
                for gq in range(NGQ):
                    blocks = range(gq * GQ, min((gq + 1) * GQ, NB))
                    csum = group_csa(lay, gq)
                    gt = {}
                    for part in (0, 1):
                        if csum[part] == 0:
                            continue
                        n = csum[part] * P
                        ist = bp.tile([P, n // 16], I16, tag=f"ist{part}")
                        nc.sync.dma_start(
                            ist[:],
                            is_d[(lay, part)][:, col[part]:col[part] + n // 16])
                        col[part] += n // 16
                        tbl = table[:] if part == 0 else table[LO_LIMIT:, :]
                        g = bp.tile([P, csum[part], rowlen], DT8, tag=f"g{part}")
                        nc.gpsimd.dma_gather(
                            g[:], tbl, ist[:], n, n, rowlen, single_packet=False)
                        gt[part] = g

                    off = {0: 0, 1: 0}
                    offa = 0
                    for b in blocks:
                        eng = nc.vector
                        cap0 = int(lo_c[b])
                        cap1 = int(hi_c[b])
                        nchb = cap0 + cap1
                        rhs = sp.tile([P, nchb, 4 + DL], DT2, tag="rhs")
                        o = 0
                        for part, cap in ((0, cap0), (1, cap1)):
                            if cap == 0:
                                continue
                            g = gt[part][:, off[part]:off[part] + cap, :]
                            adsl = adc[gq][:, offa + o:offa + o + cap, :]
                            lg = sp.tile([P, cap, H], F32, tag="lg")
                            nc.vector.tensor_tensor(
                                out=lg[:],
                                in0=g[:, :, DL:DL + asw].bitcast(F32),
                                in1=adsl, op=ALU.add)
                            lr2 = sp.tile([P, cap, H], F32, tag="lr2")
                            nc.vector.tensor_scalar_mul(lr2[:], lg[:], 0.2)
                            nc.vector.tensor_tensor(
                                out=lg[:], in0=lg[:], in1=lr2[:], op=ALU.max)
                            nc.scalar.activation(
                                rhs[:, o:o + cap, 0:4], lg[:], ACTF.Exp)
                            eng.tensor_tensor(
                                out=rhs[:, o:o + cap, 4:4 + DL].rearrange(
                                    "p c (h d) -> p c h d", h=H),
                                in0=g[:, :, 0:DL].rearrange(
                                    "p c (h d) -> p c h d", h=H),
                                in1=rhs[:, o:o + cap, 0:4][:, :, :, None].broadcast_to(
                                    [P, cap, H, DL // H]),
                                op=ALU.mult)
                            off[part] += cap
                            o += cap
                        # one-hot dst-slot matrix, per chunk: tensor_scalar
                        # is_equal against a per-partition dl scalar keeps all
                        # tensor operands 2-byte/packed (DVE fast mode)
                        S = sp.tile([P, nchb, P], DT2, tag="S")
                        for j in range(nchb):
                            nc.vector.tensor_scalar(
                                S[:, j, :], iota_sb[:],
                                dl_sb[:, ch + j:ch + j + 1], None,
                                ALU.is_equal)
                        ch += nchb
                        offa += nchb
                        ps1 = bpp.tile([P, 4 + DL], F32, tag="ps1")
                        for j in range(nchb):
                            nc.tensor.matmul(
                                ps1[:], S[:, j, :], rhs[:, j, :],
                                start=(j == 0), stop=(j == nchb - 1))
                        postproc(b, ps1)
                    if interleave is not None:
                        interleave(gq)

            # ---- ad gather pools spanning phases B and D -------------------
            with tc.tile_pool(name="ADG", bufs=2) as adgp, \
                 tc.tile_pool(name="ADC", bufs=1) as adcp:
              # ----- Phase B: layer-1 edges + layer-2 dense -------------------
              with tc.tile_pool(name="B", bufs=2) as bp, \
                 tc.tile_pool(name="Bs", bufs=3) as sp, \
                 tc.tile_pool(name="Bp", bufs=3, space="PSUM") as bpp, \
                 tc.tile_pool(name="Bp2", bufs=2, space="PSUM") as bpp2:

                h2acc = {"t": None}
                ad2acc = {"a": None}

                def post1(b, ps1):
                    dn = sp.tile([P, H], F32, tag="dn")
                    nc.vector.tensor_scalar_add(dn[:], ps1[:, 0:H], 1e-16)
                    rc = sp.tile([P, H], F32, tag="rc")
                    nc.vector.reciprocal(rc[:], dn[:])
                    o1 = sp.tile([P, D1], F32, tag="o1")
                    nc.vector.tensor_tensor(
                        out=o1[:].rearrange("p (h d) -> p h d", h=H),
                        in0=ps1[:, H:H + D1].rearrange("p (h d) -> p h d", h=H),
                        in1=rc[:][:, :, None].broadcast_to([P, H, cfg.HID]),
                        op=ALU.mult)
                    # sfull = elu(o1)+1 = min(exp(o1),1) + relu(o1)
                    exf = sp.tile([P, D1], F32, tag="exf")
                    nc.scalar.activation(exf[:], o1[:], ACTF.Exp)
                    exm = sp.tile([P, D1], DT2, tag="exm")
                    nc.vector.tensor_scalar_min(exm[:], exf[:], 1.0)
                    r1 = sp.tile([P, D1], DT2, tag="r1")
                    nc.scalar.activation(r1[:], o1[:], ACTF.Relu)
                    sfull = sp.tile([P, D1], DT2, tag="sfull")
                    nc.vector.tensor_tensor(
                        out=sfull[:], in0=exm[:], in1=r1[:], op=ALU.add)
                    ps2 = bpp2.tile([P, W2C], F32, tag="ps2")
                    for k in range(K2):
                        pt = bpp2.tile([P, P], DT2, tag="pt")
                        nc.tensor.transpose(
                            pt[:], sfull[:, k * P:(k + 1) * P], ident_sb[:])
                        st = sp.tile([P, P], DT2, tag="st")
                        nc.scalar.copy(st[:], pt[:])
                        nc.tensor.matmul(ps2[:], st[:], w2_sb[k][:],
                                         start=(k == 0), stop=False)
                    nc.tensor.matmul(ps2[:], ones1_sb[:], bias2_sb[:],
                                     start=False, stop=True)
                    # h2 row: [h2 fp8 | a_s2 f32 bitcast]; batched per AG group
                    ag = b // AGB
                    br = b - ag * AGB
                    nb_g = min(AGB, NB - ag * AGB)
                    gq2 = b // GQ
                    bq = b - gq2 * GQ
                    nb_q = min(GQ, NB - gq2 * GQ)
                    if br == 0:
                        h2acc["t"] = bp.tile([P, nb_g, D2 + 4 * H], DT8,
                                             tag="h2acc", name="h2acc")
                    if bq == 0:
                        ad2acc["a"] = bp.tile([P, nb_q, H], DT2, tag="ad2acc",
                                              name="ad2acc")
                    nc.scalar.copy(h2acc["t"][:, br, 0:D2], ps2[:, 0:D2])
                    nc.vector.tensor_copy(
                        h2acc["t"][:, br, D2:D2 + 4 * H].bitcast(F32),
                        ps2[:, D2:D2 + H])
                    nc.vector.tensor_copy(
                        ad2acc["a"][:, bq, :], ps2[:, D2 + H:D2 + 2 * H])
                    if bq == nb_q - 1:
                        nc.sync.dma_start(
                            ad2g[gq2][0:nb_q * P, 0:H].rearrange(
                                "(t p) c -> p t c", t=nb_q), ad2acc["a"][:])
                    # pipelined AllGather once this AG group's blocks are done
                    if br == nb_g - 1:
                        rows = nb_g * P
                        gbase = int(prep.agbase[ag])
                        nc.sync.dma_start(
                            h2sh[ag][0:rows, 0:D2 + 4 * H].rearrange(
                                "(t p) c -> p t c", t=nb_g), h2acc["t"][:])
                        if collective:
                            nc.gpsimd.collective_compute(
                                "AllGather", ALU.bypass,
                                replica_groups=[list(range(NCORES))],
                                ins=[h2sh[ag][0:rows, :].opt()],
                                outs=[table2[gbase:gbase + NCORES * rows,
                                             :].opt()],
                            )
                        else:
                            for r in range(NCORES):
                                base = gbase + r * rows
                                nc.sync.dma_start(
                                    table2[base:base + rows, :],
                                    h2sh[ag][0:rows, :])

                # L1 ad gathers pipelined with lookahead 2: enough to stay
                # ahead of block processing without front-loading all the ad
                # traffic before the first src gather on the DMA device
                LOOK = 2
                adc1 = [None] * NGQ
                ad2c = [None] * NGQ
                for g in range(min(LOOK, NGQ)):
                    adc1[g] = prebuild_ad(1, ad1, g,
                                          sum(group_csa(1, g).values()),
                                          adgp, adcp)

                def interleave(gq):
                    if gq + LOOK < NGQ:
                        g = gq + LOOK
                        adc1[g] = prebuild_ad(1, ad1, g,
                                              sum(group_csa(1, g).values()),
                                              adgp, adcp)
                    # even D-ad groups ride along in B (lagged one group so the
                    # ad2g flush is complete); odd groups gather in phase D
                    for g in ([gq - 1] if gq < NGQ - 1 else [gq - 1, gq]):
                        if g >= 0 and g % 2 == 0 and ad2c[g] is None:
                            ad2c[g] = prebuild_ad(2, ad2g[g], g,
                                                  sum(group_csa(2, g).values()),
                                                  adgp, adcp)

                edge_layer(1, table1, dl1_sb, D1, post1, bp, sp, bpp, adc1,
                           interleave=interleave)

              # --------------- Phase D: layer-2 edges -----------------------
              with tc.tile_pool(name="D", bufs=3) as bp, \
                 tc.tile_pool(name="Ds", bufs=3) as sp, \
                 tc.tile_pool(name="Dp", bufs=3, space="PSUM") as bpp:

                def post2(b, ps1):
                    dn = sp.tile([P, H], F32, tag="dn")
                    nc.vector.tensor_scalar_add(dn[:], ps1[:, 0:H], 1e-16)
                    rc = sp.tile([P, H], F32, tag="rc")
                    nc.vector.reciprocal(rc[:], dn[:])
                    o2 = sp.tile([P, D2], F32, tag="o2")
                    nc.vector.tensor_tensor(
                        out=o2[:].rearrange("p (h d) -> p h d", h=H),
                        in0=ps1[:, H:H + D2].rearrange("p (h d) -> p h d", h=H),
                        in1=rc[:][:, :, None].broadcast_to([P, H, cfg.LABELS]),
                        op=ALU.mult)
                    # sigmoid(x) = 1/(1+exp(-x)): Exp stays in act set 0
                    en = sp.tile([P, D2], F32, tag="en")
                    nc.scalar.activation(en[:], o2[:], ACTF.Exp, scale=-1.0)
                    nc.vector.tensor_scalar_add(en[:], en[:], 1.0)
                    sg = sp.tile([P, D2], F32, tag="sg")
                    nc.vector.reciprocal(sg[:], en[:])
                    nc.sync.dma_start(out[b * P:(b + 1) * P, :], sg[:])

                # remaining (odd) D ad gathers: first ones fill the
                # collective-tail bubble, later ones interleave with src
                for g in range(min(4, NGQ)):
                    if ad2c[g] is None:
                        ad2c[g] = prebuild_ad(2, ad2g[g], g,
                                              sum(group_csa(2, g).values()),
                                              adgp, adcp)

                def interleave2(gq):
                    g = gq + LOOK
                    if g < NGQ and ad2c[g] is None:
                        ad2c[g] = prebuild_ad(2, ad2g[g], g,
                                              sum(group_csa(2, g).values()),
                                              adgp, adcp)

                edge_layer(2, table2, dl2_sb, D2, post2, bp, sp, bpp, ad2c,
                           interleave=interleave2)

    nc.compile()
    return nc


def make_inputs(cfg, prep, x, W1, att_src1, att_dst1, b1, W2, att_src2,
                att_dst2, b2):
    """Per-core in_maps for the SPMD program."""
    H, HID, LB = cfg.HEADS, cfg.HID, cfg.LABELS
    D1, D2 = cfg.D1, cfg.D2
    W1 = np.asarray(W1, np.float32)
    W2 = np.asarray(W2, np.float32)
    as1 = np.asarray(att_src1, np.float32)
    ad1 = np.asarray(att_dst1, np.float32)
    as2 = np.asarray(att_src2, np.float32)
    ad2 = np.asarray(att_dst2, np.float32)
    b1 = np.asarray(b1, np.float32)
    b2 = np.asarray(b2, np.float32)

    A_s1 = np.einsum("ihc,hc->ih", W1.reshape(-1, H, HID), as1)
    A_d1 = np.einsum("ihc,hc->ih", W1.reshape(-1, H, HID), ad1)
    w1e = np.concatenate([W1, A_s1, A_d1], axis=1).astype(np.float16)
    # bias1 = [b1 | b1·att_s1 | b1·att_d1]
    b1h = b1.reshape(H, HID)
    bias1_row = np.concatenate(
        [b1, np.einsum("hc,hc->h", b1h, as1), np.einsum("hc,hc->h", b1h, ad1)]
    ).astype(np.float32)[None, :]

    A_s2 = np.einsum("ihc,hc->ih", W2.reshape(-1, H, LB), as2)
    A_d2 = np.einsum("ihc,hc->ih", W2.reshape(-1, H, LB), ad2)
    w2e_f = np.concatenate([W2, A_s2, A_d2], axis=1)
    b2h = b2.reshape(H, LB)
    bias2_row = (np.concatenate(
        [b2, np.einsum("hc,hc->h", b2h, as2), np.einsum("hc,hc->h", b2h, ad2)])
                 - w2e_f.sum(axis=0)).astype(np.float32)[None, :]
    w2e = w2e_f.astype(np.float16)

    iota = np.tile(np.arange(P, dtype=np.float16), (P, 1))
    ident = np.eye(P, dtype=np.float32).astype(np.float16)
    ones1 = np.ones((1, P), np.float32)

    # global position-ordered xT, then per-core rotation
    x = np.asarray(x, np.float32)
    xg = np.zeros((cfg.NPOS, cfg.IN_F), np.float32)
    xg[prep.gpos] = x
    xTg = np.ascontiguousarray(xg.T).astype(np.float16)

    in_maps = []
    for c in range(NCORES):
        xTc = np.ascontiguousarray(np.roll(xTg, -c * cfg.own_pad, axis=1))
        m = {
            "xT": xTc,
            "w1e": w1e, "w2e": w2e,
            "bias1": bias1_row, "bias2": bias2_row,
            "ones1": ones1, "iota": iota, "ident": ident,
            "dz": np.zeros((P, 8), np.int16),
            "dl1": prep.dstloc[(1, c)], "dl2": prep.dstloc[(2, c)],
            "ia1": prep.idx_ad[(1, c)], "ia2": prep.idx_ad[(2, c)],
        }
        for lay in (1, 2):
            for part in (0, 1):
                a = prep.idx_src[(lay, c, part)]
                if a.shape[1] == 0:
                    a = np.zeros((P, 16), np.int16)
                m[f"is{lay}p{part}"] = a
        in_maps.append(m)
    return in_maps, bool(np.any(b1 != 0))


def assemble_output(cfg, prep, results):
    big = np.concatenate([results[c]["out"] for c in range(NCORES)], axis=0)
    return np.ascontiguousarray(big[prep.gpos]).astype(np.float32)


_CACHE = {}


def _get_program(cfg, prep, with_bias1):
    key = (cfg.N, cfg.IN_F, cfg.HEADS, cfg.HID, cfg.LABELS, with_bias1,
           tuple(prep.caps[1][0]), tuple(prep.caps[1][1]),
           tuple(prep.caps[2][0]), tuple(prep.caps[2][1]))
    if key not in _CACHE:
        _CACHE[key] = build_program(cfg, prep, with_bias1)
    return _CACHE[key]


def kernel(x, edge_index, W1, att_src1, att_dst1, b1, W2, att_src2, att_dst2,
           b2):
    x = np.asarray(x)
    cfg = Cfg(n_nodes=x.shape[0], in_f=x.shape[1],
              hid=np.asarray(att_src1).shape[1], heads=np.asarray(att_src1).shape[0],
              labels=np.asarray(att_src2).shape[1])
    prep = HostPrep(cfg, np.asarray(edge_index))
    in_maps, with_bias1 = make_inputs(cfg, prep, x, W1, att_src1, att_dst1,
                                      b1, W2, att_src2, att_dst2, b2)
    nc = _get_program(cfg, prep, with_bias1)
    res = run_bass_kernel_spmd(nc, in_maps, core_ids=list(range(NCORES)))
    return assemble_output(cfg, prep, res.results)

